# revision 1
# baseline (speedup 1.0000x reference)
"""APPNP GNN kernel for 8 Trainium2 NeuronCores.

Strategy (per sharding hint): nodes sharded across 8 cores; edges
partitioned by destination core so segment_sum is local; per
propagation step the (pre-scaled) z shards are AllGathered, then each
core gathers source rows with SWDGE dma_gather and accumulates into
its local aggregate with dma_scatter_add.  All graph structure
(indices, chunking) is baked into the NEFF at build time; the only
runtime tensors are x / W1 / b1 / W2 / b2 (+ structure-derived
constant coefficient vectors).

SWDGE descriptor generation is the bottleneck: each dma_gather /
dma_scatter_add instruction is serviced by the Q7 core pair selected
by its queue_num (cpu_id/2 == queue_num), so all chunks on one queue
serialize on 2 of the 8 GpSimd cores.  Chunks are therefore spread
over all 4 SWDGE queues.  Correctness of concurrent scatter-adds is
kept by making queues dst-disjoint: local dst d goes to queue d % 4,
so no two queues ever RMW the same aggregate row.
"""

import math
import os
import sys

import numpy as np

sys.path.insert(0, "/opt/trn_rl_repo")

NCORES = 8
NQ = 4  # SWDGE queues (ucode MAX_SWDGE_QUEUES)
BLK = 32768  # int16-addressable row window for dma_gather
G = 1024  # edges per SWDGE instruction (64 descs/lane = 1-packet max)

# full-size problem constants (hardcoded per problem spec)
N_FULL = 100_000
E_FULL = 3_200_000
F_IN = 512
HID = 64
COUT = 64
K_STEPS = 10
ALPHA = 0.1


def _plan(N, F, C, K, src, dst):
    """Host-side structural preprocessing -> per-core tensors + schedule."""
    NSH = N // NCORES
    # padded shard, multiple of 512, strictly > NSH so the last NQ rows are
    # padding rows usable as per-queue dummy-scatter targets
    PSH = ((NSH + 512) // 512) * 512
    NP = PSH * NCORES
    nblk = (NP + BLK - 1) // BLK

    deg = np.bincount(dst, minlength=N).astype(np.float64) + 1.0
    dinv = (1.0 / np.sqrt(deg)).astype(np.float32)

    core_of = dst // NSH
    # per (core, queue, block) int16 gather/scatter index lists
    gi_mqb = [[[None] * nblk for _ in range(NQ)] for _ in range(NCORES)]
    si_mqb = [[[None] * nblk for _ in range(NQ)] for _ in range(NCORES)]
    maxdeg = np.zeros((NQ, nblk), dtype=np.int64)
    maxcnt = np.zeros((NQ, nblk), dtype=np.int64)
    for m in range(NCORES):
        sel = np.nonzero(core_of == m)[0]
        s = src[sel]
        d = (dst[sel] - m * NSH).astype(np.int64)
        sp = (s // NSH) * PSH + (s % NSH)  # padded global src id
        blk = sp // BLK
        q_of = d % NQ  # dst-class -> queue (dst-disjoint across queues)
        for q in range(NQ):
            for b in range(nblk):
                bm = (blk == b) & (q_of == q)
                gi = (sp[bm] - b * BLK).astype(np.int16)
                si = d[bm].astype(np.int16)
                # sort by dst (groups same-dst edges for the chunk coloring)
                o = np.argsort(si, kind="stable")
                gi_mqb[m][q][b] = gi[o]
                si_mqb[m][q][b] = si[o]
                maxcnt[q, b] = max(maxcnt[q, b], len(si))
                if len(si):
                    maxdeg[q, b] = max(
                        maxdeg[q, b], np.bincount(si[o].astype(np.int64)).max())

    # uniform chunk schedule across cores (same NEFF on all 8).
    # dma_scatter_add loses updates when one instruction carries duplicate
    # dst indices (RMW race across SDMA engines), so every real dst must
    # appear at most once per chunk: nch >= max per-dst degree, and each
    # dst's edges are spread round-robin over chunks.
    nch = [[max(1, int(maxdeg[q][b]), int((maxcnt[q][b] + G - 1) // G))
            for b in range(nblk)] for q in range(NQ)]
    colored = [[[None] * nblk for _ in range(NQ)] for _ in range(NCORES)]
    for q in range(NQ):
        for b in range(nblk):
            while True:
                ok = True
                for m in range(NCORES):
                    gi = gi_mqb[m][q][b]
                    si = si_mqb[m][q][b]
                    nc_b = nch[q][b]
                    chunks_g = [[] for _ in range(nc_b)]
                    chunks_s = [[] for _ in range(nc_b)]
                    if len(si):
                        si64 = si.astype(np.int64)
                        grp_start = np.r_[0, np.nonzero(np.diff(si64))[0] + 1]
                        j_in_grp = np.arange(len(si64)) - np.repeat(
                            grp_start, np.diff(np.r_[grp_start, len(si64)]))
                        cid = (si64 + j_in_grp) % nc_b
                        for c in range(nc_b):
                            cm = cid == c
                            chunks_g[c] = gi[cm]
                            chunks_s[c] = si[cm]
                        if max(len(cg) for cg in chunks_g) > G:
                            ok = False
                            break
                    colored[m][q][b] = (chunks_g, chunks_s)
                if ok:
                    break
                nch[q][b] += 1
    tot_ch = [sum(nch[q]) for q in range(NQ)]

    # pack idx arrays per queue: [128, tot_ch*(G//16)] int16 -- chunk ci at
    # cols ci*(G//16)..; each chunk's [16, G//16] block (element i of chunk
    # at [i%16, i//16]) is replicated 8x down the partitions (one copy per
    # GPSIMD Q7 core, per the dma_gather contract)
    gidx_t = [np.zeros((NCORES, 128, tot_ch[q] * (G // 16)), dtype=np.int16)
              for q in range(NQ)]
    sidx_t = [np.zeros((NCORES, 128, tot_ch[q] * (G // 16)), dtype=np.int16)
              for q in range(NQ)]
    for m in range(NCORES):
        for q in range(NQ):
            ci = 0
            for b in range(nblk):
                chunks_g, chunks_s = colored[m][q][b]
                for c in range(nch[q][b]):
                    gi = np.asarray(chunks_g[c], dtype=np.int16)
                    si = np.asarray(chunks_s[c], dtype=np.int16)
                    # sort chunk by src row for HBM locality
                    o = np.argsort(gi, kind="stable")
                    gi, si = gi[o], si[o]
                    # pad with dummy pairs: gather row 0 of this block (real,
                    # finite data), scatter to this queue's pad row (never
                    # output; duplicate dummies race only within the queue)
                    pad = G - len(gi)
                    gi = np.concatenate([gi, np.zeros(pad, dtype=np.int16)])
                    si = np.concatenate(
                        [si, np.full(pad, PSH - NQ + q, dtype=np.int16)])
                    gc = gi.reshape(G // 16, 16).T  # [16, G/16]
                    sc = si.reshape(G // 16, 16).T
                    col = ci * (G // 16)
                    gidx_t[q][m, :, col:col + G // 16] = np.tile(gc, (8, 1))
                    sidx_t[q][m, :, col:col + G // 16] = np.tile(sc, (8, 1))
                    ci += 1

    # per-node coefficient vectors, tile-major [128, PSH/128]
    T = PSH // 128
    dinv_t = np.zeros((NCORES, 128, T), dtype=np.float32)
    avec_t = np.zeros((NCORES, 128, T), dtype=np.float32)
    bvec_t = np.zeros((NCORES, 128, T), dtype=np.float32)
    for m in range(NCORES):
        dl = np.zeros(PSH, dtype=np.float32)
        dl[:NSH] = dinv[m * NSH:(m + 1) * NSH]
        dinv_t[m] = dl.reshape(T, 128).T
        avec_t[m] = (0.9 * dl).reshape(T, 128).T
        bvec_t[m] = (0.9 * dl * dl).reshape(T, 128).T

    return dict(
        NSH=NSH, PSH=PSH, NP=NP, nblk=nblk, nch=nch, tot_ch=tot_ch, T=T,
        gidx=gidx_t, sidx=sidx_t, dinv=dinv_t, avec=avec_t, bvec=bvec_t,
    )


def _build(plan, F, C, K):
    """Build the SPMD Bass program (same NEFF on all 8 cores).

    Uses several sequential TileContexts: each context allocates and
    frees its own semaphores, keeping cumulative semaphore thresholds
    under the 16-bit ISA limit.  Persistent state lives in raw SBUF
    tensors / Internal DRAM tensors that outlive the contexts.
    """
    from concourse import bacc, bass, mybir, tile
    from concourse.masks import make_identity

    PSH, NP, nblk = plan["PSH"], plan["NP"], plan["nblk"]
    T = plan["T"]
    nch, tot_ch = plan["nch"], plan["tot_ch"]
    f32 = mybir.dt.float32
    i16 = mybir.dt.int16
    KT = F // 128  # contraction tiles for W1
    NB = PSH // 512  # 512-node MLP blocks

    nc = bacc.Bacc("TRN2", target_bir_lowering=False, debug=False,
                   num_devices=NCORES, num_swdge_queues=NQ)

    xT_d = nc.dram_tensor("xT", [F, PSH], f32, kind="ExternalInput").ap()
    w1t_d = nc.dram_tensor("W1T", [F, HID], f32, kind="ExternalInput").ap()
    b1_d = nc.dram_tensor("b1c", [HID, 1], f32, kind="ExternalInput").ap()
    w2t_d = nc.dram_tensor("W2T", [HID, C], f32, kind="ExternalInput").ap()
    b2_d = nc.dram_tensor("b2c", [C, 1], f32, kind="ExternalInput").ap()
    dinv_d = nc.dram_tensor("dinv", [128, T], f32, kind="ExternalInput").ap()
    avec_d = nc.dram_tensor("avec", [128, T], f32, kind="ExternalInput").ap()
    bvec_d = nc.dram_tensor("bvec", [128, T], f32, kind="ExternalInput").ap()
    gidx_d = [nc.dram_tensor(f"gidx{q}", [128, tot_ch[q] * (G // 16)], i16,
                             kind="ExternalInput").ap() for q in range(NQ)]
    sidx_d = [nc.dram_tensor(f"sidx{q}", [128, tot_ch[q] * (G // 16)], i16,
                             kind="ExternalInput").ap() for q in range(NQ)]
    out_d = nc.dram_tensor("out", [PSH, C], f32, kind="ExternalOutput").ap()

    # persistent DRAM scratch
    zs_shard = nc.dram_tensor("zs_shard", [PSH, C], f32, kind="Internal").ap()
    zs_full = nc.dram_tensor("zs_full", [nblk * BLK, C], f32,
                             kind="Internal").ap()
    agg_dr = nc.dram_tensor("agg_dr", [PSH, C], f32, kind="Internal").ap()

    # persistent SBUF state + constants (outlive the TileContexts)
    z_sb = nc.alloc_sbuf_tensor("z_sb", [128, T, C], f32).ap()
    h01_sb = nc.alloc_sbuf_tensor("h01_sb", [128, T, C], f32).ap()
    zs_sb = nc.alloc_sbuf_tensor("zs_sb", [128, T, C], f32).ap()
    agg_sb = nc.alloc_sbuf_tensor("agg_sb", [128, T, C], f32).ap()
    w1t_sb = nc.alloc_sbuf_tensor("w1t_sb", [128, KT, HID], f32).ap()
    w2t_sb = nc.alloc_sbuf_tensor("w2t_sb", [HID, C], f32).ap()
    b1_sb = nc.alloc_sbuf_tensor("b1_sb", [HID, 1], f32).ap()
    b2_sb = nc.alloc_sbuf_tensor("b2_sb", [C, 1], f32).ap()
    dinv_sb = nc.alloc_sbuf_tensor("dinv_sb", [128, T], f32).ap()
    avec_sb = nc.alloc_sbuf_tensor("avec_sb", [128, T], f32).ap()
    bvec_sb = nc.alloc_sbuf_tensor("bvec_sb", [128, T], f32).ap()
    ident = nc.alloc_sbuf_tensor("ident", [128, 128], f32).ap()
    zero_sb = nc.alloc_sbuf_tensor("zero_sb", [128, 64], f32).ap()

    dinv_b = dinv_sb.unsqueeze(2).to_broadcast([128, T, C])
    avec_b = avec_sb.unsqueeze(2).to_broadcast([128, T, C])
    bvec_b = bvec_sb.unsqueeze(2).to_broadcast([128, T, C])
    zsf_dst = zs_shard.rearrange("(t p) c -> p t c", p=128)
    agg_src = agg_dr.rearrange("(t p) c -> p t c", p=128)

    # ---- context 1: constants + MLP ----
    with tile.TileContext(nc) as tc:
        with (
            tc.tile_pool(name="xin", bufs=2) as xin,
            tc.tile_pool(name="mlps", bufs=2) as mlps,
            tc.tile_pool(name="psum", bufs=2, space="PSUM") as psum,
            tc.tile_pool(name="psumt", bufs=2, space="PSUM") as psumt,
        ):
            for t in range(KT):
                nc.sync.dma_start(w1t_sb[:, t, :], w1t_d[t * 128:(t + 1) * 128, :])
            nc.sync.dma_start(w2t_sb, w2t_d[:])
            nc.sync.dma_start(b1_sb, b1_d[:])
            nc.sync.dma_start(b2_sb, b2_d[:])
            nc.sync.dma_start(dinv_sb, dinv_d[:])
            nc.sync.dma_start(avec_sb, avec_d[:])
            nc.sync.dma_start(bvec_sb, bvec_d[:])
            make_identity(nc, ident)
            nc.vector.memset(zero_sb, 0.0)

            for nb in range(NB):
                xb = xin.tile([128, KT, 512], f32, tag="xb")
                for t in range(KT):
                    nc.sync.dma_start(
                        xb[:, t, :],
                        xT_d[t * 128:(t + 1) * 128, nb * 512:(nb + 1) * 512],
                    )
                ph = psum.tile([HID, 512], f32, tag="ph")
                for t in range(KT):
                    nc.tensor.matmul(ph[:], w1t_sb[:, t, :], xb[:, t, :],
                                     start=(t == 0), stop=(t == KT - 1))
                hT = mlps.tile([HID, 512], f32, tag="hT")
                nc.scalar.activation(hT[:], ph[:],
                                     mybir.ActivationFunctionType.Relu,
                                     bias=b1_sb[:, :1], scale=1.0)
                ph2 = psum.tile([C, 512], f32, tag="ph2")
                nc.tensor.matmul(ph2[:], w2t_sb, hT[:], start=True, stop=True)
                h2T = mlps.tile([C, 512], f32, tag="h2T")
                nc.scalar.activation(h2T[:], ph2[:],
                                     mybir.ActivationFunctionType.Copy,
                                     bias=0.0, scale=1.0)
                nc.vector.tensor_scalar_add(h2T[:], h2T[:], b2_sb[:, :1])
                for j in range(4):
                    pt = psumt.tile([128, C], f32, tag="pt")
                    nc.tensor.transpose(pt[:], h2T[:, j * 128:(j + 1) * 128],
                                        ident[:C, :C])
                    tt = nb * 4 + j
                    nc.vector.tensor_copy(z_sb[:, tt, :], pt[:])
                    nc.scalar.activation(h01_sb[:, tt, :], pt[:],
                                         mybir.ActivationFunctionType.Copy,
                                         bias=0.0, scale=ALPHA)

    # per-queue flat chunk schedule: chunk position -> block
    flat = [[b for b in range(nblk) for _ in range(nch[q][b])]
            for q in range(NQ)]
    col0 = []  # per queue: block -> starting column in idx arrays
    for q in range(NQ):
        cols, acc = [], 0
        for b in range(nblk):
            cols.append(acc)
            acc += nch[q][b] * (G // 16)
        col0.append(cols)
    max_pos = max(len(f) for f in flat)

    # ---- propagation: one context per STEPS_PER_CTX steps ----
    SPC = 2
    for s0 in range(0, K, SPC):
        with tile.TileContext(nc) as tc:
            with tc.tile_pool(name="gat", bufs=2) as gat:
                for s in range(s0, min(s0 + SPC, K)):
                    nc.vector.tensor_tensor(zs_sb, z_sb, dinv_b,
                                            op=mybir.AluOpType.mult)
                    nc.sync.dma_start(zsf_dst, zs_sb)
                    nc.gpsimd.collective_compute(
                        "AllGather", mybir.AluOpType.bypass,
                        replica_groups=[list(range(NCORES))],
                        ins=[zs_shard.opt()],
                        outs=[zs_full[:NP, :].opt()],
                    )
                    nc.sync.dma_start(
                        agg_src,
                        zero_sb.unsqueeze(1).to_broadcast([128, T, C]),
                    )
                    gi_t = [None] * NQ
                    si_t = [None] * NQ
                    blk_cur = [-1] * NQ
                    coff = [0] * NQ  # chunk offset within current block
                    for pos in range(max_pos):
                        for q in range(NQ):
                            if pos >= len(flat[q]):
                                continue
                            b = flat[q][pos]
                            if b != blk_cur[q]:
                                ncols = nch[q][b] * (G // 16)
                                gi_t[q] = gat.tile([128, ncols], i16,
                                                   name=f"gi_t{q}",
                                                   tag=f"gi{q}", bufs=1)
                                nc.sync.dma_start(
                                    gi_t[q][:],
                                    gidx_d[q][:, col0[q][b]:col0[q][b] + ncols])
                                si_t[q] = gat.tile([128, ncols], i16,
                                                   name=f"si_t{q}",
                                                   tag=f"si{q}", bufs=1)
                                nc.sync.dma_start(
                                    si_t[q][:],
                                    sidx_d[q][:, col0[q][b]:col0[q][b] + ncols])
                                blk_cur[q] = b
                                coff[q] = 0
                            cc = coff[q] * (G // 16)
                            gt = gat.tile([128, G // 128, C], f32,
                                          tag=f"gt{q}", bufs=2)
                            nc.gpsimd.dma_gather(
                                gt[:],
                                zs_full[b * BLK:(b + 1) * BLK, :],
                                gi_t[q][:, cc:cc + G // 16],
                                G, G, C, queue_num=q,
                            )
                            nc.gpsimd.dma_scatter_add(
                                agg_dr[:],
                                gt[:],
                                si_t[q][:, cc:cc + G // 16],
                                G, G, C, queue_num=q,
                            )
                            coff[q] += 1
                    nc.sync.dma_start(agg_sb, agg_src)
                    nc.vector.tensor_tensor(agg_sb, agg_sb, avec_b,
                                            op=mybir.AluOpType.mult)
                    nc.vector.tensor_tensor(z_sb, z_sb, bvec_b,
                                            op=mybir.AluOpType.mult)
                    nc.vector.tensor_tensor(z_sb, z_sb, agg_sb,
                                            op=mybir.AluOpType.add)
                    nc.vector.tensor_tensor(z_sb, z_sb, h01_sb,
                                            op=mybir.AluOpType.add)

    # ---- final context: log_softmax + output ----
    with tile.TileContext(nc) as tc:
        with tc.tile_pool(name="fin", bufs=1) as fin:
            red = fin.tile([128, T, 1], f32)
            nc.vector.tensor_reduce(red[:], z_sb,
                                    axis=mybir.AxisListType.X,
                                    op=mybir.AluOpType.max)
            nc.vector.tensor_tensor(z_sb, z_sb,
                                    red[:].to_broadcast([128, T, C]),
                                    op=mybir.AluOpType.subtract)
            nc.scalar.activation(zs_sb, z_sb,
                                 mybir.ActivationFunctionType.Exp,
                                 bias=0.0, scale=1.0)
            nc.vector.tensor_reduce(red[:], zs_sb,
                                    axis=mybir.AxisListType.X,
                                    op=mybir.AluOpType.add)
            lse = fin.tile([128, T, 1], f32)
            nc.scalar.activation(lse[:], red[:],
                                 mybir.ActivationFunctionType.Ln,
                                 bias=0.0, scale=1.0)
            nc.vector.tensor_tensor(z_sb, z_sb,
                                    lse[:].to_broadcast([128, T, C]),
                                    op=mybir.AluOpType.subtract)
            nc.sync.dma_start(out_d.rearrange("(t p) c -> p t c", p=128),
                              z_sb)

    nc.compile()
    return nc


_CACHE = {}


def _get_compiled(key, plan, F, C, K):
    if key not in _CACHE:
        _CACHE[key] = _build(plan, F, C, K)
    return _CACHE[key]


def run(x, W1, b1, W2, b2, edge_index, N, E, F, C, K, trace=False):
    from concourse import bass_utils

    src = np.asarray(edge_index[0], dtype=np.int64)
    dst = np.asarray(edge_index[1], dtype=np.int64)
    plan = _plan(N, F, C, K, src, dst)
    NSH, PSH = plan["NSH"], plan["PSH"]

    nc = _get_compiled((N, E, F, C, K, G), plan, F, C, K)

    x = np.asarray(x, dtype=np.float32)
    xT = np.ascontiguousarray(x.T)  # [F, N]
    W1T = np.ascontiguousarray(np.asarray(W1, dtype=np.float32).T)
    W2T = np.ascontiguousarray(np.asarray(W2, dtype=np.float32).T)
    b1c = np.asarray(b1, dtype=np.float32).reshape(HID, 1)
    b2c = np.asarray(b2, dtype=np.float32).reshape(COUT, 1)

    in_maps = []
    for m in range(NCORES):
        xTs = np.zeros((F, PSH), dtype=np.float32)
        xTs[:, :NSH] = xT[:, m * NSH:(m + 1) * NSH]
        im = {
            "xT": xTs, "W1T": W1T, "b1c": b1c, "W2T": W2T, "b2c": b2c,
            "dinv": plan["dinv"][m], "avec": plan["avec"][m],
            "bvec": plan["bvec"][m],
        }
        for q in range(NQ):
            im[f"gidx{q}"] = plan["gidx"][q][m]
            im[f"sidx{q}"] = plan["sidx"][q][m]
        in_maps.append(im)

    try:
        res = bass_utils.run_bass_kernel_spmd(
            nc, in_maps, core_ids=list(range(NCORES)), trace=trace,
        )
    except ModuleNotFoundError:
        res = bass_utils.run_bass_kernel_spmd(
            nc, in_maps, core_ids=list(range(NCORES)), trace=False,
        )
    outs = res.results
    full = np.empty((N, C), dtype=np.float32)
    for m in range(NCORES):
        full[m * NSH:(m + 1) * NSH] = outs[m]["out"][:NSH]
    return full, res


def kernel(x, W1, b1, W2, b2, edge_index):
    out, _ = run(x, W1, b1, W2, b2, edge_index,
                 N=N_FULL, E=E_FULL, F=F_IN, C=COUT, K=K_STEPS)
    return out



# revision 16
# speedup vs baseline: 2.0781x; 2.0781x over previous
"""APPNP GNN kernel for 8 Trainium2 NeuronCores.

Strategy: nodes sharded across 8 cores; edges partitioned by destination
core and sorted by destination.  Per propagation step the z shards are
AllGathered to HBM; each core then SWDGE-gathers source rows in
dst-sorted order (4096 indices per instruction) and computes the
weighted segment-sum on the TensorEngine: each 128-edge chunk is a
matmul with the gathered tile [128, 64] as stationary and a
host-precomputed coefficient matrix [128, width] (norm value at
(edge_row, dst_col)) as moving operand, accumulating into a PSUM window
of 512 destination columns.  Windows are opened by a zeroing matmul so
chunks may split destinations arbitrarily; gather sources are split
into 4 blocks of <=32768 rows for int16 index addressing, accumulating
into the same windows.  No scatter-add, no per-edge descriptor RMW.

Chunk column ranges are chosen by a cross-core merged walk (cut when
any core would exceed 128 edges) so all 8 cores share one SPMD NEFF;
per-core structure lives in runtime idx/coefficient tables.
"""

import math
import sys

import ml_dtypes

import numpy as np

sys.path.insert(0, "/opt/trn_rl_repo")

NCORES = 8
NQ = 4       # SWDGE queues
BLK = 32768  # int16-addressable row window for dma_gather
GI = 1024    # gather idxs per SWDGE instruction (64 descs/lane 1-packet max)
CPI = GI // 128  # chunks per gather instruction
WIN = 512    # psum window columns

# full-size problem constants (hardcoded per problem spec)
N_FULL = 100_000
E_FULL = 3_200_000
F_IN = 512
HID = 64
COUT = 64
K_STEPS = 10
ALPHA = 0.1


def _plan(N, F, C, K, src, dst):
    """Host-side structural preprocessing -> shared schedule + per-core
    tables."""
    NSH = N // NCORES
    PSH = ((NSH + 511) // 512) * 512
    NP = PSH * NCORES
    HSH = PSH // 2          # half-shard rows (AllGather split in 2)
    NPH = HSH * NCORES      # rows per half gather-source tensor
    nhb = (NPH + BLK - 1) // BLK  # blocks per half
    nblk = 2 * nhb          # total gather groups (half, block)
    NWIN = PSH // WIN
    T = PSH // 128

    deg = np.bincount(dst, minlength=N).astype(np.float64) + 1.0
    dinv = (1.0 / np.sqrt(deg)).astype(np.float64)

    core_of = dst // NSH
    val_all = (0.9 * dinv[src] * dinv[dst]).astype(np.float32)
    # half/row in the half-gather tensors: shard row l -> half l//HSH,
    # row (src_core)*HSH + l%HSH
    l_all = src % NSH
    half_all = l_all // HSH
    sp_all = (src // NSH) * HSH + (l_all % HSH)  # row within half tensor
    b_all = half_all * nhb + sp_all // BLK       # gather group
    r_all = sp_all % BLK                         # block-relative row

    # per (core, w, b): edge lists sorted by dst col
    # counts[m][w*nblk+b] histogram over WIN cols for the merged walk
    percore = []  # m -> dict[(w,b)] -> (cols, rows, vals) sorted by col
    cnt = np.zeros((NCORES, NWIN * nblk, WIN), dtype=np.int64)
    for m in range(NCORES):
        sel = np.nonzero(core_of == m)[0]
        d = dst[sel] - m * NSH
        w = d // WIN
        c = d % WIN
        b = b_all[sel]
        r = r_all[sel]
        v = val_all[sel]
        key = (w * nblk + b) * WIN + c
        o = np.argsort(key, kind="stable")
        keys, cs, rs, vs = key[o], c[o], r[o], v[o]
        grp = keys // WIN  # w*nblk+b
        np.add.at(cnt[m].reshape(-1), keys, 1)
        # group boundaries per (w,b)
        bounds = np.searchsorted(grp, np.arange(NWIN * nblk + 1))
        tabs = {}
        for g in range(NWIN * nblk):
            s0, s1 = bounds[g], bounds[g + 1]
            tabs[g] = (cs[s0:s1], rs[s0:s1], vs[s0:s1])
        percore.append(tabs)

    # merged walk -> shared chunk col ranges per (w, b)
    ranges = {}  # g=(w*nblk+b) -> list[(c0, c1)]
    for g in range(NWIN * nblk):
        col_counts = cnt[:, g, :]  # [NCORES, WIN]
        assert col_counts.max() <= 128, "single dst col exceeds 128 edges"
        lst = []
        run = np.zeros(NCORES, dtype=np.int64)
        c0 = 0
        for c in range(WIN):
            cc = col_counts[:, c]
            if (run + cc).max() > 128:
                lst.append((c0, c))
                c0 = c
                run = cc.copy()
            else:
                run += cc
        if run.max() > 0 or c0 < WIN:
            lst.append((c0, WIN))
        # drop chunks that are empty on all cores
        lst2 = []
        for (a, bb) in lst:
            if col_counts[:, a:bb].sum() > 0:
                lst2.append((a, bb))
        ranges[g] = lst2

    # per-block chunk streams (window-major), instruction packing
    chunks_b = []   # b -> list of (w, c0, c1)
    for b in range(nblk):
        lst = []
        for w in range(NWIN):
            for (c0, c1) in ranges[w * nblk + b]:
                lst.append((w, c0, c1))
        chunks_b.append(lst)
    ninstr = [max(1, (len(chunks_b[b]) + CPI - 1) // CPI) for b in range(nblk)]
    # seg col offsets: per-instr start + per-chunk within-instr offset
    seg0 = []   # b -> [ninstr_b] start cols in seg table
    segw = []   # b -> [ninstr_b] total width per instr
    chunk_meta = []  # b -> per chunk (ii, slot, so)
    for b in range(nblk):
        s0l, swl, metas = [], [], []
        off = 0
        for ii in range(ninstr[b]):
            s0l.append(off)
            tot = 0
            for j in range(CPI):
                k = ii * CPI + j
                if k >= len(chunks_b[b]):
                    break
                wdt = chunks_b[b][k][2] - chunks_b[b][k][1]
                metas.append((ii, j, tot))
                tot += wdt
            swl.append(max(1, tot))
            off += max(1, tot)
        seg0.append(s0l)
        segw.append(swl)
        chunk_meta.append(metas)
    segtot = [seg0[b][-1] + segw[b][-1] for b in range(nblk)]

    # window-major matmul schedule: (w, b) -> [(ii, slot, so, c0, c1)]
    sched = {}
    for b in range(nblk):
        ptr = {}
        for k, (w, c0, c1) in enumerate(chunks_b[b]):
            ii, j, so = chunk_meta[b][k]
            sched.setdefault((w, b), []).append((ii, j, so, c0, c1))

    # per-core runtime tables
    gidx = [np.zeros((NCORES, 128, ninstr[b] * (GI // 16)), dtype=np.int16)
            for b in range(nblk)]
    seg = [np.zeros((NCORES, 128, segtot[b]), dtype=ml_dtypes.bfloat16)
           for b in range(nblk)]
    rows128 = np.arange(128)
    for m in range(NCORES):
        tabs = percore[m]
        for b in range(nblk):
            for k, (w, c0, c1) in enumerate(chunks_b[b]):
                cs, rs, vs = tabs[w * nblk + b]
                lo = np.searchsorted(cs, c0)
                hi = np.searchsorted(cs, c1)
                n = hi - lo
                assert n <= 128
                r = np.zeros(128, dtype=np.int16)
                cc = np.zeros(128, dtype=np.int64)
                vv = np.zeros(128, dtype=np.float32)
                if n:
                    rr, ccx, vvx = rs[lo:hi], cs[lo:hi] - c0, vs[lo:hi]
                    o = np.argsort(rr, kind="stable")
                    r[:n] = rr[o]
                    cc[:n] = ccx[o]
                    vv[:n] = vvx[o]
                ii, j, so = chunk_meta[b][k]
                # idx layout: position p in instr -> [p%16 (+16*rep), p//16]
                p = j * 128 + rows128
                col = ii * (GI // 16) + p // 16
                row = p % 16
                for rep in range(8):
                    gidx[b][m, row + 16 * rep, col] = r
                seg[b][m, rows128, seg0[b][ii] + so + cc] = vv.astype(
                    ml_dtypes.bfloat16)

    # self-loop coefficient vector, tile-major [128, T]
    bvec_t = np.zeros((NCORES, 128, T), dtype=np.float32)
    for m in range(NCORES):
        dl = np.zeros(PSH, dtype=np.float64)
        dl[:NSH] = dinv[m * NSH:(m + 1) * NSH]
        bvec_t[m] = (0.9 * dl * dl).astype(np.float32).reshape(T, 128).T

    return dict(
        NSH=NSH, PSH=PSH, NP=NP, HSH=HSH, NPH=NPH, nhb=nhb, nblk=nblk,
        NWIN=NWIN, T=T,
        ninstr=ninstr, seg0=seg0, segw=segw, segtot=segtot, sched=sched,
        gidx=gidx, seg=seg, bvec=bvec_t,
    )


def _build(plan, F, C, K):
    """Build the SPMD Bass program (same NEFF on all 8 cores)."""
    from concourse import bacc, bass, mybir, tile
    from concourse.masks import make_identity

    PSH, NP, nblk = plan["PSH"], plan["NP"], plan["nblk"]
    HSH, NPH, nhb = plan["HSH"], plan["NPH"], plan["nhb"]
    NWIN, T = plan["NWIN"], plan["T"]
    ninstr, seg0, segw, segtot = (plan["ninstr"], plan["seg0"],
                                  plan["segw"], plan["segtot"])
    sched = plan["sched"]
    f32 = mybir.dt.float32
    f16 = mybir.dt.bfloat16
    i16 = mybir.dt.int16
    ES = 2 * C  # gathered row elems (bf16, 256B incl. pad)
    KT = F // 128
    NB = PSH // 512

    nc = bacc.Bacc("TRN2", target_bir_lowering=False, debug=False,
                   num_devices=NCORES, num_swdge_queues=NQ)

    xT_d = nc.dram_tensor("xT", [F, PSH], f32, kind="ExternalInput").ap()
    w1t_d = nc.dram_tensor("W1T", [F, HID], f32, kind="ExternalInput").ap()
    b1_d = nc.dram_tensor("b1c", [HID, 1], f32, kind="ExternalInput").ap()
    w2t_d = nc.dram_tensor("W2T", [HID, C], f32, kind="ExternalInput").ap()
    b2_d = nc.dram_tensor("b2c", [C, 1], f32, kind="ExternalInput").ap()
    bvec_d = nc.dram_tensor("bvec", [128, T], f32, kind="ExternalInput").ap()
    gidx_d = [nc.dram_tensor(f"gidx{b}", [128, ninstr[b] * (GI // 16)], i16,
                             kind="ExternalInput").ap() for b in range(nblk)]
    seg_d = [nc.dram_tensor(f"seg{b}", [128, segtot[b]], f16,
                            kind="ExternalInput").ap() for b in range(nblk)]
    out_d = nc.dram_tensor("out", [PSH, C], f32, kind="ExternalOutput").ap()

    zs_shard = nc.dram_tensor("zs_shard", [PSH, ES], f16,
                              kind="Internal").ap()
    zs_half = [nc.dram_tensor(f"zs_half{h}", [NPH, ES], f16,
                              kind="Internal").ap() for h in range(2)]

    # persistent SBUF state + constants
    z_sb = nc.alloc_sbuf_tensor("z_sb", [128, T, C], f32).ap()
    h01_sb = nc.alloc_sbuf_tensor("h01_sb", [128, T, C], f32).ap()
    agg_sb = nc.alloc_sbuf_tensor("agg_sb", [128, T, C], f32).ap()
    w1t_sb = nc.alloc_sbuf_tensor("w1t_sb", [128, KT, HID], f32).ap()
    w2t_sb = nc.alloc_sbuf_tensor("w2t_sb", [HID, C], f32).ap()
    b1_sb = nc.alloc_sbuf_tensor("b1_sb", [HID, 1], f32).ap()
    b2_sb = nc.alloc_sbuf_tensor("b2_sb", [C, 1], f32).ap()
    bvec_sb = nc.alloc_sbuf_tensor("bvec_sb", [128, T], f32).ap()
    ident = nc.alloc_sbuf_tensor("ident", [128, 128], f32).ap()
    ident16 = nc.alloc_sbuf_tensor("ident16", [128, 128], f16).ap()
    zseg = nc.alloc_sbuf_tensor("zseg", [128, WIN], f16).ap()
    zb16_sb = nc.alloc_sbuf_tensor("zb16_sb", [128, T, C], f16).ap()

    bvec_b = bvec_sb.unsqueeze(2).to_broadcast([128, T, C])
    TH = T // 2
    zsf_dst = [
        zs_shard[:HSH, :C].rearrange("(t p) c -> p t c", p=128),
        zs_shard[HSH:, :C].rearrange("(t p) c -> p t c", p=128),
    ]
    blk_ap = [zs_half[g // nhb][(g % nhb) * BLK:
                                min(NPH, (g % nhb + 1) * BLK), :]
              for g in range(nblk)]

    # ---- context 1: constants + MLP ----
    with tile.TileContext(nc) as tc:
        with (
            tc.tile_pool(name="xin", bufs=2) as xin,
            tc.tile_pool(name="mlps", bufs=2) as mlps,
            tc.tile_pool(name="psum", bufs=2, space="PSUM") as psum,
            tc.tile_pool(name="psumt", bufs=2, space="PSUM") as psumt,
        ):
            for t in range(KT):
                nc.sync.dma_start(w1t_sb[:, t, :], w1t_d[t * 128:(t + 1) * 128, :])
            nc.sync.dma_start(w2t_sb, w2t_d[:])
            nc.sync.dma_start(b1_sb, b1_d[:])
            nc.sync.dma_start(b2_sb, b2_d[:])
            nc.sync.dma_start(bvec_sb, bvec_d[:])
            make_identity(nc, ident)
            nc.vector.tensor_copy(ident16, ident)
            nc.vector.memset(zseg, 0.0)

            for nb in range(NB):
                xb = xin.tile([128, KT, 512], f32, tag="xb")
                for t in range(KT):
                    nc.sync.dma_start(
                        xb[:, t, :],
                        xT_d[t * 128:(t + 1) * 128, nb * 512:(nb + 1) * 512],
                    )
                ph = psum.tile([HID, 512], f32, tag="ph")
                for t in range(KT):
                    nc.tensor.matmul(ph[:], w1t_sb[:, t, :], xb[:, t, :],
                                     start=(t == 0), stop=(t == KT - 1))
                hT = mlps.tile([HID, 512], f32, tag="hT")
                nc.scalar.activation(hT[:], ph[:],
                                     mybir.ActivationFunctionType.Relu,
                                     bias=b1_sb[:, :1], scale=1.0)
                ph2 = psum.tile([C, 512], f32, tag="ph2")
                nc.tensor.matmul(ph2[:], w2t_sb, hT[:], start=True, stop=True)
                h2T = mlps.tile([C, 512], f32, tag="h2T")
                nc.scalar.activation(h2T[:], ph2[:],
                                     mybir.ActivationFunctionType.Copy,
                                     bias=0.0, scale=1.0)
                nc.vector.tensor_scalar_add(h2T[:], h2T[:], b2_sb[:, :1])
                for j in range(4):
                    pt = psumt.tile([128, C], f32, tag="pt")
                    nc.tensor.transpose(pt[:], h2T[:, j * 128:(j + 1) * 128],
                                        ident[:C, :C])
                    tt = nb * 4 + j
                    nc.vector.tensor_copy(z_sb[:, tt, :], pt[:])
                    nc.scalar.activation(h01_sb[:, tt, :], pt[:],
                                         mybir.ActivationFunctionType.Copy,
                                         bias=0.0, scale=ALPHA)

    # ---- propagation ----
    SPC = 2
    qctr = 0
    for s0 in range(0, K, SPC):
        with tile.TileContext(nc) as tc:
            with (
                tc.tile_pool(name="gat", bufs=3) as gat,
                tc.tile_pool(name="segp", bufs=3) as segp,
                tc.tile_pool(name="pws", bufs=2, space="PSUM") as pws,
                tc.tile_pool(name="ptp", bufs=2, space="PSUM") as ptp,
                tc.tile_pool(name="stg", bufs=2) as stg,
            ):
                for s in range(s0, min(s0 + SPC, K)):
                    nc.vector.tensor_copy(zb16_sb, z_sb)
                    for h in range(2):
                        nc.sync.dma_start(zsf_dst[h],
                                          zb16_sb[:, h * TH:(h + 1) * TH, :])
                        nc.gpsimd.collective_compute(
                            "AllGather", mybir.AluOpType.bypass,
                            replica_groups=[list(range(NCORES))],
                            ins=[(zs_shard[:HSH] if h == 0
                                  else zs_shard[HSH:]).opt()],
                            outs=[zs_half[h][:].opt()],
                        )
                    cur_ii = [-1] * nblk
                    gt_t = [None] * nblk
                    st_t = [None] * nblk
                    for w in range(NWIN):
                        pw = pws.tile([C, WIN], f32, tag="pw")
                        total_wb = sum(len(sched.get((w, b), []))
                                       for b in range(nblk))
                        nc.tensor.matmul(pw[:], ident16[:, :C], zseg[:],
                                         start=True, stop=(total_wb == 0))
                        done = 0
                        for b in range(nblk):
                            for (ii, j, so, c0, c1) in sched.get((w, b), []):
                                if ii != cur_ii[b]:
                                    git = gat.tile([128, GI // 16], i16,
                                                   tag=f"gi{b}")
                                    nc.sync.dma_start(
                                        git[:],
                                        gidx_d[b][:, ii * (GI // 16):
                                                  (ii + 1) * (GI // 16)])
                                    gt = gat.tile([128, CPI, ES], f16,
                                                  tag=f"gt{b}")
                                    nc.gpsimd.dma_gather(
                                        gt[:], blk_ap[b], git[:], GI, GI, ES,
                                        queue_num=b % NQ)
                                    st = segp.tile([128, segw[b][ii]], f16,
                                                   tag=f"st{b}")
                                    nc.sync.dma_start(
                                        st[:],
                                        seg_d[b][:, seg0[b][ii]:
                                                 seg0[b][ii] + segw[b][ii]])
                                    cur_ii[b] = ii
                                    gt_t[b], st_t[b] = gt, st
                                done += 1
                                nc.tensor.matmul(
                                    pw[:, c0:c1], gt_t[b][:, j, :C],
                                    st_t[b][:, so:so + (c1 - c0)],
                                    start=False, stop=(done == total_wb))
                        sg = stg.tile([C, WIN], f32, tag="sg")
                        nc.scalar.activation(sg[:], pw[:],
                                             mybir.ActivationFunctionType.Copy,
                                             bias=0.0, scale=1.0)
                        for jj in range(WIN // 128):
                            pt = ptp.tile([128, C], f32, tag="pt")
                            nc.tensor.transpose(pt[:],
                                                sg[:, jj * 128:(jj + 1) * 128],
                                                ident[:C, :C])
                            nc.vector.tensor_copy(
                                agg_sb[:, w * (WIN // 128) + jj, :], pt[:])
                    # combine: z = agg + bvec*z + h01
                    nc.vector.tensor_tensor(z_sb, z_sb, bvec_b,
                                            op=mybir.AluOpType.mult)
                    nc.vector.tensor_tensor(z_sb, z_sb, agg_sb,
                                            op=mybir.AluOpType.add)
                    nc.vector.tensor_tensor(z_sb, z_sb, h01_sb,
                                            op=mybir.AluOpType.add)

    # ---- final context: log_softmax + output ----
    with tile.TileContext(nc) as tc:
        with tc.tile_pool(name="fin", bufs=1) as fin:
            red = fin.tile([128, T, 1], f32)
            nc.vector.tensor_reduce(red[:], z_sb,
                                    axis=mybir.AxisListType.X,
                                    op=mybir.AluOpType.max)
            nc.vector.tensor_tensor(z_sb, z_sb,
                                    red[:].to_broadcast([128, T, C]),
                                    op=mybir.AluOpType.subtract)
            nc.scalar.activation(agg_sb, z_sb,
                                 mybir.ActivationFunctionType.Exp,
                                 bias=0.0, scale=1.0)
            nc.vector.tensor_reduce(red[:], agg_sb,
                                    axis=mybir.AxisListType.X,
                                    op=mybir.AluOpType.add)
            lse = fin.tile([128, T, 1], f32)
            nc.scalar.activation(lse[:], red[:],
                                 mybir.ActivationFunctionType.Ln,
                                 bias=0.0, scale=1.0)
            nc.vector.tensor_tensor(z_sb, z_sb,
                                    lse[:].to_broadcast([128, T, C]),
                                    op=mybir.AluOpType.subtract)
            nc.sync.dma_start(out_d.rearrange("(t p) c -> p t c", p=128),
                              z_sb)

    nc.compile()
    return nc


_CACHE = {}


def _get_compiled(key, plan, F, C, K):
    if key not in _CACHE:
        _CACHE[key] = _build(plan, F, C, K)
    return _CACHE[key]


def _make_in_maps(plan, x, W1, b1, W2, b2, F):
    NSH, PSH, nblk = plan["NSH"], plan["PSH"], plan["nblk"]
    x = np.asarray(x, dtype=np.float32)
    xT = np.ascontiguousarray(x.T)
    W1T = np.ascontiguousarray(np.asarray(W1, dtype=np.float32).T)
    W2T = np.ascontiguousarray(np.asarray(W2, dtype=np.float32).T)
    b1c = np.asarray(b1, dtype=np.float32).reshape(HID, 1)
    b2c = np.asarray(b2, dtype=np.float32).reshape(COUT, 1)

    in_maps = []
    for m in range(NCORES):
        xTs = np.zeros((F, PSH), dtype=np.float32)
        xTs[:, :NSH] = xT[:, m * NSH:(m + 1) * NSH]
        im = {
            "xT": xTs, "W1T": W1T, "b1c": b1c, "W2T": W2T, "b2c": b2c,
            "bvec": plan["bvec"][m],
        }
        for b in range(nblk):
            im[f"gidx{b}"] = plan["gidx"][b][m]
            im[f"seg{b}"] = plan["seg"][b][m]
        in_maps.append(im)
    return in_maps


def run(x, W1, b1, W2, b2, edge_index, N, E, F, C, K, trace=False):
    from concourse import bass_utils

    src = np.asarray(edge_index[0], dtype=np.int64)
    dst = np.asarray(edge_index[1], dtype=np.int64)
    plan = _plan(N, F, C, K, src, dst)
    NSH = plan["NSH"]

    nc = _get_compiled((N, E, F, C, K, GI), plan, F, C, K)
    in_maps = _make_in_maps(plan, x, W1, b1, W2, b2, F)

    try:
        res = bass_utils.run_bass_kernel_spmd(
            nc, in_maps, core_ids=list(range(NCORES)), trace=trace,
        )
    except ModuleNotFoundError:
        res = bass_utils.run_bass_kernel_spmd(
            nc, in_maps, core_ids=list(range(NCORES)), trace=False,
        )
    outs = res.results
    full = np.empty((N, C), dtype=np.float32)
    for m in range(NCORES):
        full[m * NSH:(m + 1) * NSH] = outs[m]["out"][:NSH]
    return full, res


def kernel(x, W1, b1, W2, b2, edge_index):
    out, _ = run(x, W1, b1, W2, b2, edge_index,
                 N=N_FULL, E=E_FULL, F=F_IN, C=COUT, K=K_STEPS)
    return out


# revision 17
# speedup vs baseline: 2.3848x; 1.1476x over previous
"""APPNP GNN kernel for 8 Trainium2 NeuronCores.

Strategy: nodes sharded across 8 cores; edges partitioned by destination
core and sorted by destination.  Per propagation step the z shards are
AllGathered to HBM; each core then SWDGE-gathers source rows in
dst-sorted order (4096 indices per instruction) and computes the
weighted segment-sum on the TensorEngine: each 128-edge chunk is a
matmul with the gathered tile [128, 64] as stationary and a
host-precomputed coefficient matrix [128, width] (norm value at
(edge_row, dst_col)) as moving operand, accumulating into a PSUM window
of 512 destination columns.  Windows are opened by a zeroing matmul so
chunks may split destinations arbitrarily; gather sources are split
into 4 blocks of <=32768 rows for int16 index addressing, accumulating
into the same windows.  No scatter-add, no per-edge descriptor RMW.

Chunk column ranges are chosen by a cross-core merged walk (cut when
any core would exceed 128 edges) so all 8 cores share one SPMD NEFF;
per-core structure lives in runtime idx/coefficient tables.
"""

import math
import sys

import ml_dtypes

import numpy as np

sys.path.insert(0, "/opt/trn_rl_repo")

NCORES = 8
NQ = 4       # SWDGE queues
BLK = 32768  # int16-addressable row window for dma_gather
GI = 1024    # gather idxs per SWDGE instruction (64 descs/lane 1-packet max)
CPI = GI // 128  # chunks per gather instruction
WIN = 512    # psum window columns

# full-size problem constants (hardcoded per problem spec)
N_FULL = 100_000
E_FULL = 3_200_000
F_IN = 512
HID = 64
COUT = 64
K_STEPS = 10
ALPHA = 0.1


def _plan(N, F, C, K, src, dst):
    """Host-side structural preprocessing -> shared schedule + per-core
    tables."""
    NSH = N // NCORES
    PSH = ((NSH + 511) // 512) * 512
    NP = PSH * NCORES
    HSH = PSH // 2          # half-shard rows (AllGather split in 2)
    NPH = HSH * NCORES      # rows per half gather-source tensor
    nhb = (NPH + BLK - 1) // BLK  # blocks per half
    nblk = 2 * nhb          # total gather groups (half, block)
    NWIN = PSH // WIN
    T = PSH // 128

    deg = np.bincount(dst, minlength=N).astype(np.float64) + 1.0
    dinv = (1.0 / np.sqrt(deg)).astype(np.float64)

    core_of = dst // NSH
    val_all = (0.9 * dinv[src] * dinv[dst]).astype(np.float32)
    # half/row in the half-gather tensors: shard row l -> half l//HSH,
    # row (src_core)*HSH + l%HSH
    l_all = src % NSH
    half_all = l_all // HSH
    sp_all = (src // NSH) * HSH + (l_all % HSH)  # row within half tensor
    b_all = half_all * nhb + sp_all // BLK       # gather group
    r_all = sp_all % BLK                         # block-relative row

    # per (core, w, b): edge lists sorted by dst col
    # counts[m][w*nblk+b] histogram over WIN cols for the merged walk
    percore = []  # m -> dict[(w,b)] -> (cols, rows, vals) sorted by col
    cnt = np.zeros((NCORES, NWIN * nblk, WIN), dtype=np.int64)
    for m in range(NCORES):
        sel = np.nonzero(core_of == m)[0]
        d = dst[sel] - m * NSH
        w = d // WIN
        c = d % WIN
        b = b_all[sel]
        r = r_all[sel]
        v = val_all[sel]
        key = (w * nblk + b) * WIN + c
        o = np.argsort(key, kind="stable")
        keys, cs, rs, vs = key[o], c[o], r[o], v[o]
        grp = keys // WIN  # w*nblk+b
        np.add.at(cnt[m].reshape(-1), keys, 1)
        # group boundaries per (w,b)
        bounds = np.searchsorted(grp, np.arange(NWIN * nblk + 1))
        tabs = {}
        for g in range(NWIN * nblk):
            s0, s1 = bounds[g], bounds[g + 1]
            tabs[g] = (cs[s0:s1], rs[s0:s1], vs[s0:s1])
        percore.append(tabs)

    # merged walk -> shared chunk col ranges per (w, b)
    ranges = {}  # g=(w*nblk+b) -> list[(c0, c1)]
    for g in range(NWIN * nblk):
        col_counts = cnt[:, g, :]  # [NCORES, WIN]
        assert col_counts.max() <= 128, "single dst col exceeds 128 edges"
        lst = []
        run = np.zeros(NCORES, dtype=np.int64)
        c0 = 0
        for c in range(WIN):
            cc = col_counts[:, c]
            if (run + cc).max() > 128:
                lst.append((c0, c))
                c0 = c
                run = cc.copy()
            else:
                run += cc
        if run.max() > 0 or c0 < WIN:
            lst.append((c0, WIN))
        # drop chunks that are empty on all cores
        lst2 = []
        for (a, bb) in lst:
            if col_counts[:, a:bb].sum() > 0:
                lst2.append((a, bb))
        ranges[g] = lst2

    # per-block chunk streams (window-major), instruction packing
    chunks_b = []   # b -> list of (w, c0, c1)
    for b in range(nblk):
        lst = []
        for w in range(NWIN):
            for (c0, c1) in ranges[w * nblk + b]:
                lst.append((w, c0, c1))
        chunks_b.append(lst)
    ninstr = [max(1, (len(chunks_b[b]) + CPI - 1) // CPI) for b in range(nblk)]
    # seg col offsets: per-instr start + per-chunk within-instr offset
    seg0 = []   # b -> [ninstr_b] start cols in seg table
    segw = []   # b -> [ninstr_b] total width per instr
    chunk_meta = []  # b -> per chunk (ii, slot, so)
    for b in range(nblk):
        s0l, swl, metas = [], [], []
        off = 0
        for ii in range(ninstr[b]):
            s0l.append(off)
            tot = 0
            for j in range(CPI):
                k = ii * CPI + j
                if k >= len(chunks_b[b]):
                    break
                wdt = chunks_b[b][k][2] - chunks_b[b][k][1]
                metas.append((ii, j, tot))
                tot += wdt
            swl.append(max(1, tot))
            off += max(1, tot)
        seg0.append(s0l)
        segw.append(swl)
        chunk_meta.append(metas)
    segtot = [seg0[b][-1] + segw[b][-1] for b in range(nblk)]

    # window-major matmul schedule: (w, b) -> [(ii, slot, so, c0, c1)]
    sched = {}
    for b in range(nblk):
        ptr = {}
        for k, (w, c0, c1) in enumerate(chunks_b[b]):
            ii, j, so = chunk_meta[b][k]
            sched.setdefault((w, b), []).append((ii, j, so, c0, c1))

    # per-core runtime tables
    gidx = [np.zeros((NCORES, 128, ninstr[b] * (GI // 16)), dtype=np.int16)
            for b in range(nblk)]
    seg = [np.zeros((NCORES, 128, segtot[b]), dtype=ml_dtypes.bfloat16)
           for b in range(nblk)]
    rows128 = np.arange(128)
    for m in range(NCORES):
        tabs = percore[m]
        for b in range(nblk):
            for k, (w, c0, c1) in enumerate(chunks_b[b]):
                cs, rs, vs = tabs[w * nblk + b]
                lo = np.searchsorted(cs, c0)
                hi = np.searchsorted(cs, c1)
                n = hi - lo
                assert n <= 128
                r = np.zeros(128, dtype=np.int16)
                cc = np.zeros(128, dtype=np.int64)
                vv = np.zeros(128, dtype=np.float32)
                if n:
                    rr, ccx, vvx = rs[lo:hi], cs[lo:hi] - c0, vs[lo:hi]
                    o = np.argsort(rr, kind="stable")
                    r[:n] = rr[o]
                    cc[:n] = ccx[o]
                    vv[:n] = vvx[o]
                ii, j, so = chunk_meta[b][k]
                # idx layout: position p in instr -> [p%16 (+16*rep), p//16]
                p = j * 128 + rows128
                col = ii * (GI // 16) + p // 16
                row = p % 16
                for rep in range(8):
                    gidx[b][m, row + 16 * rep, col] = r
                seg[b][m, rows128, seg0[b][ii] + so + cc] = vv.astype(
                    ml_dtypes.bfloat16)

    # self-loop coefficient vector, tile-major [128, T]
    bvec_t = np.zeros((NCORES, 128, T), dtype=np.float32)
    for m in range(NCORES):
        dl = np.zeros(PSH, dtype=np.float64)
        dl[:NSH] = dinv[m * NSH:(m + 1) * NSH]
        bvec_t[m] = (0.9 * dl * dl).astype(np.float32).reshape(T, 128).T

    return dict(
        NSH=NSH, PSH=PSH, NP=NP, HSH=HSH, NPH=NPH, nhb=nhb, nblk=nblk,
        NWIN=NWIN, T=T,
        ninstr=ninstr, seg0=seg0, segw=segw, segtot=segtot, sched=sched,
        gidx=gidx, seg=seg, bvec=bvec_t,
    )


def _build(plan, F, C, K):
    """Build the SPMD Bass program (same NEFF on all 8 cores)."""
    from concourse import bacc, bass, mybir, tile
    from concourse.masks import make_identity

    PSH, NP, nblk = plan["PSH"], plan["NP"], plan["nblk"]
    HSH, NPH, nhb = plan["HSH"], plan["NPH"], plan["nhb"]
    NWIN, T = plan["NWIN"], plan["T"]
    ninstr, seg0, segw, segtot = (plan["ninstr"], plan["seg0"],
                                  plan["segw"], plan["segtot"])
    sched = plan["sched"]
    f32 = mybir.dt.float32
    f16 = mybir.dt.bfloat16
    i16 = mybir.dt.int16
    ES = 2 * C  # gathered row elems (bf16, 256B incl. pad)
    KT = F // 128
    NB = PSH // 512

    nc = bacc.Bacc("TRN2", target_bir_lowering=False, debug=False,
                   num_devices=NCORES, num_swdge_queues=NQ)

    xT_d = nc.dram_tensor("xT", [F, PSH], f32, kind="ExternalInput").ap()
    w1t_d = nc.dram_tensor("W1T", [F, HID], f32, kind="ExternalInput").ap()
    b1_d = nc.dram_tensor("b1c", [HID, 1], f32, kind="ExternalInput").ap()
    w2t_d = nc.dram_tensor("W2T", [HID, C], f32, kind="ExternalInput").ap()
    b2_d = nc.dram_tensor("b2c", [C, 1], f32, kind="ExternalInput").ap()
    bvec_d = nc.dram_tensor("bvec", [128, T], f32, kind="ExternalInput").ap()
    gidx_d = [nc.dram_tensor(f"gidx{b}", [128, ninstr[b] * (GI // 16)], i16,
                             kind="ExternalInput").ap() for b in range(nblk)]
    seg_d = [nc.dram_tensor(f"seg{b}", [128, segtot[b]], f16,
                            kind="ExternalInput").ap() for b in range(nblk)]
    out_d = nc.dram_tensor("out", [PSH, C], f32, kind="ExternalOutput").ap()

    zs_shard = nc.dram_tensor("zs_shard", [PSH, ES], f16,
                              kind="Internal").ap()
    zs_half = [nc.dram_tensor(f"zs_half{h}", [NPH, ES], f16,
                              kind="Internal").ap() for h in range(2)]

    # persistent SBUF state + constants
    z_sb = nc.alloc_sbuf_tensor("z_sb", [128, T, C], f32).ap()
    h01_sb = nc.alloc_sbuf_tensor("h01_sb", [128, T, C], f32).ap()
    agg_sb = nc.alloc_sbuf_tensor("agg_sb", [128, T, C], f32).ap()
    w1t_sb = nc.alloc_sbuf_tensor("w1t_sb", [128, KT, HID], f32).ap()
    w2t_sb = nc.alloc_sbuf_tensor("w2t_sb", [HID, C], f32).ap()
    b1_sb = nc.alloc_sbuf_tensor("b1_sb", [HID, 1], f32).ap()
    b2_sb = nc.alloc_sbuf_tensor("b2_sb", [C, 1], f32).ap()
    bvec_sb = nc.alloc_sbuf_tensor("bvec_sb", [128, T], f32).ap()
    ident = nc.alloc_sbuf_tensor("ident", [128, 128], f32).ap()
    ident16 = nc.alloc_sbuf_tensor("ident16", [128, 128], f16).ap()
    zseg = nc.alloc_sbuf_tensor("zseg", [128, WIN], f16).ap()
    zb16_sb = nc.alloc_sbuf_tensor("zb16_sb", [128, T, C], f16).ap()

    bvec_b = bvec_sb.unsqueeze(2).to_broadcast([128, T, C])
    TH = T // 2
    zsf_dst = [
        zs_shard[:HSH, :C].rearrange("(t p) c -> p t c", p=128),
        zs_shard[HSH:, :C].rearrange("(t p) c -> p t c", p=128),
    ]
    blk_ap = [zs_half[g // nhb][(g % nhb) * BLK:
                                min(NPH, (g % nhb + 1) * BLK), :]
              for g in range(nblk)]

    # ---- context 1: constants + MLP ----
    with tile.TileContext(nc) as tc:
        with (
            tc.tile_pool(name="xin", bufs=2) as xin,
            tc.tile_pool(name="mlps", bufs=2) as mlps,
            tc.tile_pool(name="psum", bufs=2, space="PSUM") as psum,
            tc.tile_pool(name="psumt", bufs=2, space="PSUM") as psumt,
        ):
            for t in range(KT):
                nc.sync.dma_start(w1t_sb[:, t, :], w1t_d[t * 128:(t + 1) * 128, :])
            nc.sync.dma_start(w2t_sb, w2t_d[:])
            nc.sync.dma_start(b1_sb, b1_d[:])
            nc.sync.dma_start(b2_sb, b2_d[:])
            nc.sync.dma_start(bvec_sb, bvec_d[:])
            make_identity(nc, ident)
            nc.vector.tensor_copy(ident16, ident)
            nc.vector.memset(zseg, 0.0)

            for nb in range(NB):
                xb = xin.tile([128, KT, 512], f32, tag="xb")
                for t in range(KT):
                    nc.sync.dma_start(
                        xb[:, t, :],
                        xT_d[t * 128:(t + 1) * 128, nb * 512:(nb + 1) * 512],
                    )
                ph = psum.tile([HID, 512], f32, tag="ph")
                for t in range(KT):
                    nc.tensor.matmul(ph[:], w1t_sb[:, t, :], xb[:, t, :],
                                     start=(t == 0), stop=(t == KT - 1))
                hT = mlps.tile([HID, 512], f32, tag="hT")
                nc.scalar.activation(hT[:], ph[:],
                                     mybir.ActivationFunctionType.Relu,
                                     bias=b1_sb[:, :1], scale=1.0)
                ph2 = psum.tile([C, 512], f32, tag="ph2")
                nc.tensor.matmul(ph2[:], w2t_sb, hT[:], start=True, stop=True)
                h2T = mlps.tile([C, 512], f32, tag="h2T")
                nc.scalar.activation(h2T[:], ph2[:],
                                     mybir.ActivationFunctionType.Copy,
                                     bias=0.0, scale=1.0)
                nc.vector.tensor_scalar_add(h2T[:], h2T[:], b2_sb[:, :1])
                for j in range(4):
                    pt = psumt.tile([128, C], f32, tag="pt")
                    nc.tensor.transpose(pt[:], h2T[:, j * 128:(j + 1) * 128],
                                        ident[:C, :C])
                    tt = nb * 4 + j
                    nc.vector.tensor_copy(z_sb[:, tt, :], pt[:])
                    nc.scalar.activation(h01_sb[:, tt, :], pt[:],
                                         mybir.ActivationFunctionType.Copy,
                                         bias=0.0, scale=ALPHA)

    # ---- propagation ----
    SPC = 2
    qctr = 0
    for s0 in range(0, K, SPC):
        with tile.TileContext(nc) as tc:
            with (
                tc.tile_pool(name="gat", bufs=3) as gat,
                tc.tile_pool(name="segp", bufs=3) as segp,
                tc.tile_pool(name="pws", bufs=2, space="PSUM") as pws,
                tc.tile_pool(name="ptp", bufs=2, space="PSUM") as ptp,
                tc.tile_pool(name="stg", bufs=2) as stg,
            ):
                for s in range(s0, min(s0 + SPC, K)):
                    nc.vector.tensor_copy(zb16_sb, z_sb)
                    for h in range(2):
                        nc.sync.dma_start(zsf_dst[h],
                                          zb16_sb[:, h * TH:(h + 1) * TH, :])
                        nc.gpsimd.collective_compute(
                            "AllGather", mybir.AluOpType.bypass,
                            replica_groups=[list(range(NCORES))],
                            ins=[(zs_shard[:HSH] if h == 0
                                  else zs_shard[HSH:]).opt()],
                            outs=[zs_half[h][:].opt()],
                        )
                    cur_ii = [-1] * nblk
                    gt_t = [None] * nblk
                    st_t = [None] * nblk
                    for w in range(NWIN):
                        pw = pws.tile([C, WIN], f32, tag="pw")
                        total_wb = sum(len(sched.get((w, b), []))
                                       for b in range(nblk))
                        nc.tensor.matmul(pw[:], ident16[:, :C], zseg[:],
                                         start=True, stop=(total_wb == 0))
                        done = 0
                        for b in range(nblk):
                            for (ii, j, so, c0, c1) in sched.get((w, b), []):
                                if ii != cur_ii[b]:
                                    git = gat.tile([128, GI // 16], i16,
                                                   tag=f"gi{b}")
                                    nc.sync.dma_start(
                                        git[:],
                                        gidx_d[b][:, ii * (GI // 16):
                                                  (ii + 1) * (GI // 16)])
                                    gt = gat.tile([128, CPI, ES], f16,
                                                  tag=f"gt{b}")
                                    nc.gpsimd.dma_gather(
                                        gt[:], blk_ap[b], git[:], GI, GI, ES,
                                        queue_num=qctr % NQ)
                                    qctr += 1
                                    st = segp.tile([128, segw[b][ii]], f16,
                                                   tag=f"st{b}")
                                    nc.sync.dma_start(
                                        st[:],
                                        seg_d[b][:, seg0[b][ii]:
                                                 seg0[b][ii] + segw[b][ii]])
                                    cur_ii[b] = ii
                                    gt_t[b], st_t[b] = gt, st
                                done += 1
                                nc.tensor.matmul(
                                    pw[:, c0:c1], gt_t[b][:, j, :C],
                                    st_t[b][:, so:so + (c1 - c0)],
                                    start=False, stop=(done == total_wb))
                        sg = stg.tile([C, WIN], f32, tag="sg")
                        nc.scalar.activation(sg[:], pw[:],
                                             mybir.ActivationFunctionType.Copy,
                                             bias=0.0, scale=1.0)
                        for jj in range(WIN // 128):
                            pt = ptp.tile([128, C], f32, tag="pt")
                            nc.tensor.transpose(pt[:],
                                                sg[:, jj * 128:(jj + 1) * 128],
                                                ident[:C, :C])
                            nc.vector.tensor_copy(
                                agg_sb[:, w * (WIN // 128) + jj, :], pt[:])
                    # combine: z = agg + bvec*z + h01
                    nc.vector.tensor_tensor(z_sb, z_sb, bvec_b,
                                            op=mybir.AluOpType.mult)
                    nc.vector.tensor_tensor(z_sb, z_sb, agg_sb,
                                            op=mybir.AluOpType.add)
                    nc.vector.tensor_tensor(z_sb, z_sb, h01_sb,
                                            op=mybir.AluOpType.add)

    # ---- final context: log_softmax + output ----
    with tile.TileContext(nc) as tc:
        with tc.tile_pool(name="fin", bufs=1) as fin:
            red = fin.tile([128, T, 1], f32)
            nc.vector.tensor_reduce(red[:], z_sb,
                                    axis=mybir.AxisListType.X,
                                    op=mybir.AluOpType.max)
            nc.vector.tensor_tensor(z_sb, z_sb,
                                    red[:].to_broadcast([128, T, C]),
                                    op=mybir.AluOpType.subtract)
            nc.scalar.activation(agg_sb, z_sb,
                                 mybir.ActivationFunctionType.Exp,
                                 bias=0.0, scale=1.0)
            nc.vector.tensor_reduce(red[:], agg_sb,
                                    axis=mybir.AxisListType.X,
                                    op=mybir.AluOpType.add)
            lse = fin.tile([128, T, 1], f32)
            nc.scalar.activation(lse[:], red[:],
                                 mybir.ActivationFunctionType.Ln,
                                 bias=0.0, scale=1.0)
            nc.vector.tensor_tensor(z_sb, z_sb,
                                    lse[:].to_broadcast([128, T, C]),
                                    op=mybir.AluOpType.subtract)
            nc.sync.dma_start(out_d.rearrange("(t p) c -> p t c", p=128),
                              z_sb)

    nc.compile()
    return nc


_CACHE = {}


def _get_compiled(key, plan, F, C, K):
    if key not in _CACHE:
        _CACHE[key] = _build(plan, F, C, K)
    return _CACHE[key]


def _make_in_maps(plan, x, W1, b1, W2, b2, F):
    NSH, PSH, nblk = plan["NSH"], plan["PSH"], plan["nblk"]
    x = np.asarray(x, dtype=np.float32)
    xT = np.ascontiguousarray(x.T)
    W1T = np.ascontiguousarray(np.asarray(W1, dtype=np.float32).T)
    W2T = np.ascontiguousarray(np.asarray(W2, dtype=np.float32).T)
    b1c = np.asarray(b1, dtype=np.float32).reshape(HID, 1)
    b2c = np.asarray(b2, dtype=np.float32).reshape(COUT, 1)

    in_maps = []
    for m in range(NCORES):
        xTs = np.zeros((F, PSH), dtype=np.float32)
        xTs[:, :NSH] = xT[:, m * NSH:(m + 1) * NSH]
        im = {
            "xT": xTs, "W1T": W1T, "b1c": b1c, "W2T": W2T, "b2c": b2c,
            "bvec": plan["bvec"][m],
        }
        for b in range(nblk):
            im[f"gidx{b}"] = plan["gidx"][b][m]
            im[f"seg{b}"] = plan["seg"][b][m]
        in_maps.append(im)
    return in_maps


def run(x, W1, b1, W2, b2, edge_index, N, E, F, C, K, trace=False):
    from concourse import bass_utils

    src = np.asarray(edge_index[0], dtype=np.int64)
    dst = np.asarray(edge_index[1], dtype=np.int64)
    plan = _plan(N, F, C, K, src, dst)
    NSH = plan["NSH"]

    nc = _get_compiled((N, E, F, C, K, GI), plan, F, C, K)
    in_maps = _make_in_maps(plan, x, W1, b1, W2, b2, F)

    try:
        res = bass_utils.run_bass_kernel_spmd(
            nc, in_maps, core_ids=list(range(NCORES)), trace=trace,
        )
    except ModuleNotFoundError:
        res = bass_utils.run_bass_kernel_spmd(
            nc, in_maps, core_ids=list(range(NCORES)), trace=False,
        )
    outs = res.results
    full = np.empty((N, C), dtype=np.float32)
    for m in range(NCORES):
        full[m * NSH:(m + 1) * NSH] = outs[m]["out"][:NSH]
    return full, res


def kernel(x, W1, b1, W2, b2, edge_index):
    out, _ = run(x, W1, b1, W2, b2, edge_index,
                 N=N_FULL, E=E_FULL, F=F_IN, C=COUT, K=K_STEPS)
    return out
